# revision 30
# baseline (speedup 1.0000x reference)
"""Multihead attention (B=4, S=2048, D=1024, H=16, Hd=64) on 8 trn2 cores.

Sharding: core c owns batch b = c//2 and heads [(c%2)*8, (c%2)*8+8).
Each core computes q/k/v projections for its 8 heads, attention, and the
partial output projection restricted to its heads' context features.
Host adds the two partials per batch element (+ bo).

Dtype strategy (the error gate is 2e-2; measured rel err ~6e-3):
  - x and all weights are converted to bf16 on the host; every matmul is
    bf16 x bf16 -> f32 PSUM, which runs at 1 PE-cycle per output column
    (fp32 runs at 4) at ANY moving width - that matters for the narrow
    (N=65) AV matmuls. bf16 also halves input DMA and SBUF footprint.
  - exp runs on the Act engine reading f32 PSUM scores, writing bf16 A^T
    in [128, 1024] instructions (KG=4 k-tiles per group) to amortize the
    ~185ns per-instruction access latency.

Pipeline (the graded metric is the marginal per-rep time, i.e. the
steady-state pipeline period):
  - all tile pools and weight/bias/const DMAs sit OUTSIDE the rep loop;
    consecutive reps overlap through WAR dependencies on rotating tiles.
  - q^T/k^T/vst are double-buffered (qk pool, bufs=2) so rep r+1's
    projections run inside rep r's attention window instead of blocking
    on its score/AV reads. Projections run k/q interleaved per chunk
    (per-chunk q^T/k^T tiles keep the dependencies fine-grained), then v.
  - PSUM (8 banks): proj "ps" x2 = 2, score tiles st0/st1 (2 banks each,
    KG*QC2 f32 cols) = 4, AV "ave" = 1, out-proj "p3" = 1. AV chains run
    sequentially on one bank so phase 3 gets a dedicated bank and never
    blocks the next rep's attention; the ctx transposes rotate on the p3
    bank (end-of-section work, no cross-rep hazard, and off the AV
    rotation). Four interleaved accumulation chains must NOT share one
    bank (corrupts accumulation), hence one chain at a time.
  - steady state is PE-bound: per-rep PE ~281us busy (proj 109 + scores
    109 (K=64: half-array, intrinsic to Hd=64) + AV 55 + out-proj 27);
    sim marginal ~300us/rep vs ~1130us for the fp32 baseline.

Layout:
  - inputs are fed pre-transposed (xT: [D, S]) so projection matmuls need
    no on-device transposes.
  - q, k are produced transposed ([hd, tok]); scores are computed as
    S^T = K @ Q^T with k-tokens on partitions so the exp output A^T is
    already in the layout the AV matmul needs as its stationary operand.
    Head pairs share the PE array rows (even head rows 0-63, odd 64-127).
  - AV runs with the narrow [V | 1] operand moving (N=65): out[q, 0:64] is
    the context, out[q, 64] the softmax denominator, so normalization is a
    per-partition reciprocal+scale. ctx tiles are PE-transposed into ctx^T
    for the output projection.
  - softmax skips max-subtraction: scores are ~N(0,1) here, exp is safe
    and matches the max-subtracted reference to rounding error.
"""

import numpy as np

B, S, D = 4, 2048, 1024
H, HD = 16, 64
HPC = 8              # heads per core
HF = HPC * HD        # 512 head-features per core
NCORES = 8
QC = 512             # query-chunk (matmul moving free dim)
NQC = S // QC        # 4
KT = S // 128        # 16 k-token tiles
PT = 128

_cache = {}

# exp engine split: the 256 per-rep exp instructions are distributed between
# the Act engine (native Exp, ~893ns/[128,1024]) and DVE (Schraudolph bf16-bits
# exp via tensor_scalar -> int16, ~1095ns, rms rel err ~1.8% which softmax
# normalization damps by ~sqrt(N_eff)~27x - negligible vs the 2e-2 gate).
# GPSIMD cannot read PSUM, so Pool only takes SBUF-side copies/memsets.
# Ratio ACT_N/EXP_PERIOD balances Act vs DVE (DVE also carries ~70us of
# normalization/bias work per rep).
EXP_PERIOD = 31
ACT_N = 20
SIGMA = 0.0580


def _build_nc(reps=1, ablate=()):
    """ablate: timing-only stage shrinks ("exp","scores","av","proj",
    "outproj") that keep the dependency structure but cut ~90% of one
    stage's engine work. Never used by kernel() - localizes bottlenecks."""
    from contextlib import ExitStack

    import concourse.mybir as mybir
    import concourse.tile as tile
    from concourse import bacc
    import concourse.bass as bass

    f32 = mybir.dt.float32
    bf16 = mybir.dt.bfloat16
    i16 = mybir.dt.int16
    ALU = mybir.AluOpType
    C1 = float(np.log2(np.e) / np.sqrt(HD) * 128.0)
    C2 = float((127.0 - SIGMA) * 128.0)
    nc = bacc.Bacc()

    xqT = nc.declare_dram_parameter("xqT", [D, S], bf16, isOutput=False)
    xkT = nc.declare_dram_parameter("xkT", [D, S], bf16, isOutput=False)
    xvT = nc.declare_dram_parameter("xvT", [D, S], bf16, isOutput=False)
    wqT = nc.declare_dram_parameter("wqT", [D, HF], bf16, isOutput=False)
    wkT = nc.declare_dram_parameter("wkT", [D, HF], bf16, isOutput=False)
    wvT = nc.declare_dram_parameter("wvT", [D, HF], bf16, isOutput=False)
    woT = nc.declare_dram_parameter("woT", [HF, D], bf16, isOutput=False)
    bqd = nc.declare_dram_parameter("bq", [HF], f32, isOutput=False)
    bkd = nc.declare_dram_parameter("bk", [HF], f32, isOutput=False)
    bvd = nc.declare_dram_parameter("bv", [HF], bf16, isOutput=False)
    out = nc.declare_dram_parameter("out", [S, D], f32, isOutput=True)
    identd = nc.declare_dram_parameter("ident", [PT, PT], bf16, isOutput=False)

    DKT = D // PT       # 8 feature k-tiles for projections
    QC2 = 256           # q-chunk for attention
    NQC2 = S // QC2     # 8
    KG = 2              # k-tiles per score/exp group (1 PSUM bank per tile)
    NG = KT // KG       # 8 groups

    with tile.TileContext(nc) as tc, ExitStack() as ctx:
        persist = ctx.enter_context(tc.tile_pool(name="persist", bufs=1))
        xpool = ctx.enter_context(tc.tile_pool(name="p1x", bufs=2))
        atpool = ctx.enter_context(tc.tile_pool(name="at", bufs=2))
        nrmpool = ctx.enter_context(tc.tile_pool(name="nrm", bufs=2))
        cspool = ctx.enter_context(tc.tile_pool(name="cs", bufs=2))
        opool = ctx.enter_context(tc.tile_pool(name="p3o", bufs=2))
        qkpool = ctx.enter_context(tc.tile_pool(name="qk", bufs=2))
        pspool = ctx.enter_context(tc.tile_pool(name="ps", bufs=2, space="PSUM"))
        stpool = ctx.enter_context(tc.tile_pool(name="st", bufs=1, space="PSUM"))
        avpool = ctx.enter_context(tc.tile_pool(name="av", bufs=1, space="PSUM"))
        p3pool = ctx.enter_context(tc.tile_pool(name="p3", bufs=1, space="PSUM"))

        # ---- persistent state: weights, biases, constants --------------
        ctxT = [[persist.tile([PT, PT], bf16, name=f"ctxT{i}_{t}", tag=f"ctxT{i}_{t}")
                 for t in range(KT)] for i in range(4)]
        bvb = persist.tile([PT, HF], bf16, tag="bvb")
        ident = persist.tile([PT, PT], bf16, tag="ident")
        nc.sync.dma_start(ident[:], identd[:])

        bv_ap = bvd[:]
        bv_bc_src = bass.AP(
            tensor=bv_ap.tensor, offset=bv_ap.offset, ap=[[0, PT], [1, HF]]
        )
        nc.sync.dma_start(bvb[:], bv_bc_src)

        wts = {}
        for pname, wT_d in (("v", wvT), ("k", wkT), ("q", wqT)):
            wt = persist.tile([PT, DKT, HF], bf16, name=f"w{pname}", tag=f"w{pname}")
            nc.sync.dma_start(
                wt[:], wT_d.rearrange("(k p) f -> p k f", p=PT)
            )
            wts[pname] = [wt[:, k, :] for k in range(DKT)]
        bts = {}
        for pname, b_d in (("q", bqd), ("k", bkd)):
            bt = persist.tile([PT, 4], f32, name=f"b{pname}", tag=f"b{pname}")
            nc.sync.dma_start(bt[:], b_d.rearrange("(m p) -> p m", p=PT))
            bts[pname] = [bt[:, m : m + 1] for m in range(4)]
        woTa = persist.tile([PT, 4, D], bf16, name="woT", tag="woT")
        nc.sync.dma_start(woTa[:], woT.rearrange("(i p) d -> p i d", p=PT))
        woTt = [woTa[:, i, :] for i in range(4)]

        def alloc_rep():
            # fresh double-buffered q^T/k^T/v per rep so the next rep's
            # projections never WAR-block on this rep's score reads
            qT = [[qkpool.tile([PT, QC], bf16, name=f"qT{i}_{c}", tag=f"qT{i}_{c}")
                   for c in range(NQC)] for i in range(4)]
            kTt = [[qkpool.tile([PT, QC], bf16, name=f"kT{i}_{c}", tag=f"kT{i}_{c}")
                    for c in range(NQC)] for i in range(4)]
            vst = [qkpool.tile([PT, HPC * (HD + 1)], bf16, name=f"v{t}", tag=f"v{t}")
                   for t in range(KT)]
            for t in range(KT):
                v3 = vst[t].rearrange("p (h c) -> p h c", c=HD + 1)
                nc.gpsimd.memset(v3[:, :, HD : HD + 1], 1.0)
            return qT, kTt, vst

        def make_proj_tasks(qT, kTt, vst):
            """One rep's projections as closures: one DMA or one PSUM chain
            each, emitted one-per-beat inside the PREVIOUS rep's unit stream
            so the proj phase never runs serial."""
            tasks = []
            stage = {}
            nkp = 1 if "proj" in ablate else DKT

            def dma_task(xT_d, c, key):
                def f():
                    xta = xpool.tile([PT, DKT, QC], bf16, name="x", tag="x")
                    nc.sync.dma_start(
                        xta[:],
                        xT_d.rearrange("(k p) s -> p k s", p=PT)[:, :, c * QC : (c + 1) * QC],
                    )
                    stage[key] = xta
                return f

            def kq_chain(pname, c, m, key):
                def f():
                    xta = stage[key]
                    ps = pspool.tile([PT, QC], f32, tag="ps")
                    for k in range(nkp):
                        nc.tensor.matmul(
                            ps[:],
                            lhsT=wts[pname][k][:, m * PT : (m + 1) * PT],
                            rhs=xta[:, k, :],
                            start=(k == 0),
                            stop=(k == nkp - 1),
                        )
                    dstT = kTt if pname == "k" else qT
                    nc.vector.tensor_scalar_add(
                        dstT[m][c][:], ps[:], bts[pname][m][:]
                    )
                return f

            def v_chain(c, mt, key):
                def f():
                    xta = stage[key]
                    t = c * 4 + mt
                    ps = pspool.tile([PT, HF], f32, tag="ps")
                    for k in range(nkp):
                        nc.tensor.matmul(
                            ps[:],
                            lhsT=xta[:, k, mt * PT : (mt + 1) * PT],
                            rhs=wts["v"][k][:],
                            start=(k == 0),
                            stop=(k == nkp - 1),
                        )
                    v3 = vst[t].rearrange("p (h c) -> p h c", c=HD + 1)
                    nc.vector.tensor_add(
                        v3[:, :, 0:HD],
                        ps[:].rearrange("p (h c) -> p h c", c=HD),
                        bvb[:].rearrange("p (h c) -> p h c", c=HD),
                    )
                return f

            def add_kq(c):
                for pname, xT_d in (("k", xkT), ("q", xqT)):
                    key = (pname, c)
                    tasks.append(dma_task(xT_d, c, key))
                    for m in range(4):
                        tasks.append(kq_chain(pname, c, m, key))

            def add_v(c):
                key = ("v", c)
                tasks.append(dma_task(xvT, c, key))
                for mt in range(4):
                    tasks.append(v_chain(c, mt, key))

            add_kq(0)
            for c in range(NQC):
                add_v(c)
            for c in range(1, NQC):
                add_kq(c)
            return tasks

        def emit_units(qT, kTt, vst, feed):
            # ---- phase 2+3: software-pipelined units ---------------------
            # unit u = (c, hp). Emission interleaves scores/exp of unit u with
            # the AV chains of unit u-1 in 4 "beats", so PE always has runnable
            # matmuls while Act/DVE drain the previous unit's exps. Score
            # tiles live in 4 single-bank parity slots st{e}p{g%2}; at tiles
            # have 16 tags x 2 bufs. The NEXT rep's projection tasks are fed
            # one per beat starting at unit FEED_START.
            at_store = {}
            cs_store = {}
            exp_ctr = [0]

            def emit_scores(c, hp, g):
                stp = [stpool.tile([PT, KG * QC2], f32, name=f"st{e}p{g % 2}",
                                   tag=f"st{e}p{g % 2}") for e in range(2)]
                scols = 16 if "scores" in ablate else QC2
                for j in range(KG):
                    kt = KG * g + j
                    for e in range(2):
                        nc.tensor.matmul(
                            stp[e][:, j * QC2 : j * QC2 + scols],
                            lhsT=kTt[hp][kt // 4][e * HD : (e + 1) * HD,
                                                  (kt % 4) * PT : (kt % 4 + 1) * PT],
                            rhs=qT[hp][c // 2][e * HD : (e + 1) * HD,
                                               (c % 2) * QC2 : (c % 2) * QC2 + scols],
                            start=True,
                            stop=True,
                        )
                for e in range(2):
                    a = atpool.tile([PT, KG * QC2], bf16,
                                    name=f"at{e}_{g}", tag=f"at{e}_{g}")
                    i = exp_ctr[0]
                    exp_ctr[0] += 1
                    ecols = 64 if "exp" in ablate else KG * QC2
                    if (i * ACT_N) % EXP_PERIOD < ACT_N:
                        nc.scalar.activation(
                            a[:, 0:ecols], stp[e][:, 0:ecols],
                            mybir.ActivationFunctionType.Exp,
                            scale=1.0 / np.sqrt(HD),
                        )
                    else:
                        nc.vector.tensor_scalar(
                            a[:, 0:ecols].bitcast(i16), stp[e][:, 0:ecols],
                            C1, C2, ALU.mult, ALU.add,
                        )
                    at_store[c, hp, e, g] = a

            def av_chain(c, hp, qt, e):
                # One [128, 260] PSUM tile per unit; the 4 chains use disjoint
                # 65-col regions, so subtile dep-tracking lets chain k+1
                # accumulate while DVE still normalizes chain k (no PE stall),
                # and PSUM accumulation state is per-address so interleaved
                # groups on disjoint regions are safe.
                if (c, hp, 0) not in cs_store:
                    for q2 in range(2):
                        cs_store[c, hp, q2] = cspool.tile(
                            [PT, PT], bf16, name=f"cs{hp}_{q2}", tag=f"cs{hp}_{q2}")
                    cs_store[c, hp, "ave"] = avpool.tile(
                        [PT, 4 * (HD + 1)], f32, name="av", tag="ave")
                ci = 2 * qt + e
                avp = cs_store[c, hp, "ave"][:, ci * (HD + 1) : (ci + 1) * (HD + 1)]
                h = 2 * hp + e
                nkt = 2 if "av" in ablate else KT
                for kt in range(nkt):
                    g, j = kt // KG, kt % KG
                    last = j == KG - 1 and qt == 1 and nkt == KT
                    at = at_store.pop((c, hp, e, g)) if last else at_store[c, hp, e, g]
                    nc.tensor.matmul(
                        avp,
                        lhsT=at[:, j * QC2 + qt * PT : j * QC2 + (qt + 1) * PT],
                        rhs=vst[kt][:, h * (HD + 1) : (h + 1) * (HD + 1)],
                        start=(kt == 0),
                        stop=(kt == nkt - 1),
                    )
                linv = nrmpool.tile([PT, 1], f32, tag="linv")
                nc.vector.reciprocal(linv[:], avp[:, HD : HD + 1])
                nc.vector.tensor_scalar_mul(
                    cs_store[c, hp, qt][:, e * HD : (e + 1) * HD],
                    avp[:, 0:HD],
                    linv[:],
                )
                if ci == 3:
                    cs_store.pop((c, hp, "ave"))

            def emit_tps(c, hp):
                for qt in range(2):
                    tt = 2 * c + qt
                    tp = p3pool.tile([PT, PT], bf16, name="tp", tag="p3")
                    nc.tensor.transpose(tp[:], cs_store.pop((c, hp, qt))[:], ident[:])
                    nc.vector.tensor_copy(ctxT[hp][tt][:], tp[:])

            def emit_outproj(c):
                for qt in range(2):
                    tt = 2 * c + qt
                    for nch in range(2):
                        ps = p3pool.tile([PT, QC], f32, tag="p3")
                        nko = 1 if "outproj" in ablate else 4
                        for k in range(nko):
                            nc.tensor.matmul(
                                ps[:],
                                lhsT=ctxT[k][tt][:],
                                rhs=woTt[k][:, nch * QC : (nch + 1) * QC],
                                start=(k == 0),
                                stop=(k == nko - 1),
                            )
                        ot = opool.tile([PT, QC], f32, tag="ot")
                        nc.vector.tensor_copy(ot[:], ps[:])
                        nc.sync.dma_start(
                            out[tt * PT : (tt + 1) * PT, nch * QC : (nch + 1) * QC], ot[:]
                        )

            FEED_START = 8
            units = [(c, hp) for c in range(NQC2) for hp in range(4)]
            for ui in range(len(units) + 1):
                cur = units[ui] if ui < len(units) else None
                prev = units[ui - 1] if ui >= 1 else None
                for beat in range(4):
                    if cur is not None:
                        emit_scores(cur[0], cur[1], 2 * beat)
                        emit_scores(cur[0], cur[1], 2 * beat + 1)
                    if prev is not None:
                        av_chain(prev[0], prev[1], beat // 2, beat % 2)
                    if feed and ui >= FEED_START:
                        feed.pop(0)()
                if prev is not None:
                    emit_tps(prev[0], prev[1])
                    if prev[1] == 3:
                        emit_outproj(prev[0])

        cur_tiles = alloc_rep()
        cur_tasks = make_proj_tasks(*cur_tiles)
        for t in cur_tasks:  # rep 0 prologue: projections inline
            t()
        for _rep in range(reps):
            if _rep + 1 < reps:
                nxt_tiles = alloc_rep()
                nxt_tasks = make_proj_tasks(*nxt_tiles)
            else:
                nxt_tiles, nxt_tasks = None, []
            emit_units(*cur_tiles, feed=nxt_tasks)
            for t in nxt_tasks:  # leftovers (only if feed slots ran out)
                t()
            cur_tiles = nxt_tiles

    nc.compile()
    return nc


def make_in_maps(inputs):
    import ml_dtypes

    q = np.ascontiguousarray(inputs["query"], dtype=np.float32)
    k = np.ascontiguousarray(inputs["key"], dtype=np.float32)
    v = np.ascontiguousarray(inputs["value"], dtype=np.float32)
    Wq, Wk, Wv, Wo = (np.asarray(inputs[n], np.float32) for n in ("Wq", "Wk", "Wv", "Wo"))
    bq, bk, bv, bo = (np.asarray(inputs[n], np.float32) for n in ("bq", "bk", "bv", "bo"))

    in_maps = []
    for c in range(NCORES):
        b, half = c // 2, c % 2
        fs = slice(half * HF, (half + 1) * HF)
        in_maps.append({
            "xqT": np.ascontiguousarray(q[b].T).astype(ml_dtypes.bfloat16),
            "xkT": np.ascontiguousarray(k[b].T).astype(ml_dtypes.bfloat16),
            "xvT": np.ascontiguousarray(v[b].T).astype(ml_dtypes.bfloat16),
            "wqT": np.ascontiguousarray(Wq[fs, :].T).astype(ml_dtypes.bfloat16),
            "wkT": np.ascontiguousarray(Wk[fs, :].T).astype(ml_dtypes.bfloat16),
            "wvT": np.ascontiguousarray(Wv[fs, :].T).astype(ml_dtypes.bfloat16),
            "woT": np.ascontiguousarray(Wo[:, fs].T).astype(ml_dtypes.bfloat16),
            "bq": np.ascontiguousarray(bq[fs]),
            "bk": np.ascontiguousarray(bk[fs]),
            "bv": np.ascontiguousarray(bv[fs]).astype(ml_dtypes.bfloat16),
            "ident": np.eye(PT, dtype=ml_dtypes.bfloat16),
        })
    return in_maps


def kernel(**inputs):
    from concourse.bass_utils import run_bass_kernel_spmd

    if "nc" not in _cache:
        _cache["nc"] = _build_nc()
    nc = _cache["nc"]

    in_maps = make_in_maps(inputs)
    res = run_bass_kernel_spmd(nc, in_maps, list(range(NCORES)))
    _cache["last_result"] = res

    bo = np.asarray(inputs["bo"], np.float32)
    out = np.empty((B, S, D), np.float32)
    for b in range(B):
        out[b] = res.results[2 * b]["out"] + res.results[2 * b + 1]["out"] + bo
    return out



# revision 33
# speedup vs baseline: 1.0715x; 1.0715x over previous
"""Multihead attention (B=4, S=2048, D=1024, H=16, Hd=64) on 8 trn2 cores.

Sharding: core c owns batch b = c//2 and heads [(c%2)*8, (c%2)*8+8).
Each core computes q/k/v projections for its 8 heads, attention, and the
partial output projection restricted to its heads' context features.
Host adds the two partials per batch element (+ bo).

Dtype strategy (the error gate is 2e-2; measured rel err ~6e-3):
  - x and all weights are converted to bf16 on the host; every matmul is
    bf16 x bf16 -> f32 PSUM, which runs at 1 PE-cycle per output column
    (fp32 runs at 4) at ANY moving width - that matters for the narrow
    (N=65) AV matmuls. bf16 also halves input DMA and SBUF footprint.
  - exp runs on the Act engine reading f32 PSUM scores, writing bf16 A^T
    in [128, 1024] instructions (KG=4 k-tiles per group) to amortize the
    ~185ns per-instruction access latency.

Pipeline (the graded metric is the marginal per-rep time, i.e. the
steady-state pipeline period):
  - all tile pools and weight/bias/const DMAs sit OUTSIDE the rep loop;
    consecutive reps overlap through WAR dependencies on rotating tiles.
  - q^T/k^T/vst are double-buffered (qk pool, bufs=2) so rep r+1's
    projections run inside rep r's attention window instead of blocking
    on its score/AV reads. Projections run k/q interleaved per chunk
    (per-chunk q^T/k^T tiles keep the dependencies fine-grained), then v.
  - PSUM (8 banks): proj "ps" x2 = 2, score tiles st0/st1 (2 banks each,
    KG*QC2 f32 cols) = 4, AV "ave" = 1, out-proj "p3" = 1. AV chains run
    sequentially on one bank so phase 3 gets a dedicated bank and never
    blocks the next rep's attention; the ctx transposes rotate on the p3
    bank (end-of-section work, no cross-rep hazard, and off the AV
    rotation). Four interleaved accumulation chains must NOT share one
    bank (corrupts accumulation), hence one chain at a time.
  - steady state is PE-bound: per-rep PE ~281us busy (proj 109 + scores
    109 (K=64: half-array, intrinsic to Hd=64) + AV 55 + out-proj 27);
    sim marginal ~300us/rep vs ~1130us for the fp32 baseline.

Layout:
  - inputs are fed pre-transposed (xT: [D, S]) so projection matmuls need
    no on-device transposes.
  - q, k are produced transposed ([hd, tok]); scores are computed as
    S^T = K @ Q^T with k-tokens on partitions so the exp output A^T is
    already in the layout the AV matmul needs as its stationary operand.
    Head pairs share the PE array rows (even head rows 0-63, odd 64-127).
  - AV runs with the narrow [V | 1] operand moving (N=65): out[q, 0:64] is
    the context, out[q, 64] the softmax denominator, so normalization is a
    per-partition reciprocal+scale. ctx tiles are PE-transposed into ctx^T
    for the output projection.
  - softmax skips max-subtraction: scores are ~N(0,1) here, exp is safe
    and matches the max-subtracted reference to rounding error.
"""

import numpy as np

B, S, D = 4, 2048, 1024
H, HD = 16, 64
HPC = 8              # heads per core
HF = HPC * HD        # 512 head-features per core
NCORES = 8
QC = 512             # query-chunk (matmul moving free dim)
NQC = S // QC        # 4
KT = S // 128        # 16 k-token tiles
PT = 128

_cache = {}

# exp engine split: the 256 per-rep exp instructions are distributed between
# the Act engine (native Exp, ~893ns/[128,1024]) and DVE (Schraudolph bf16-bits
# exp via tensor_scalar -> int16, ~1095ns, rms rel err ~1.8% which softmax
# normalization damps by ~sqrt(N_eff)~27x - negligible vs the 2e-2 gate).
# GPSIMD cannot read PSUM, so Pool only takes SBUF-side copies/memsets.
# Ratio ACT_N/EXP_PERIOD balances Act vs DVE (DVE also carries ~70us of
# normalization/bias work per rep).
EXP_PERIOD = 16
ACT_N = 11
SIGMA = 0.0580
FEED = "b1"   # "off" | "b1" (task/beat) | "b2" (task per 2 beats)


def _build_nc(reps=1, ablate=()):
    """ablate: timing-only stage shrinks ("exp","scores","av","proj",
    "outproj") that keep the dependency structure but cut ~90% of one
    stage's engine work. Never used by kernel() - localizes bottlenecks."""
    from contextlib import ExitStack

    import concourse.mybir as mybir
    import concourse.tile as tile
    from concourse import bacc
    import concourse.bass as bass

    f32 = mybir.dt.float32
    bf16 = mybir.dt.bfloat16
    i16 = mybir.dt.int16
    ALU = mybir.AluOpType
    C1 = float(np.log2(np.e) / np.sqrt(HD) * 128.0)
    C2 = float((127.0 - SIGMA) * 128.0)
    nc = bacc.Bacc()

    xqT = nc.declare_dram_parameter("xqT", [D, S], bf16, isOutput=False)
    xkT = nc.declare_dram_parameter("xkT", [D, S], bf16, isOutput=False)
    xvT = nc.declare_dram_parameter("xvT", [D, S], bf16, isOutput=False)
    wqT = nc.declare_dram_parameter("wqT", [D, HF], bf16, isOutput=False)
    wkT = nc.declare_dram_parameter("wkT", [D, HF], bf16, isOutput=False)
    wvT = nc.declare_dram_parameter("wvT", [D, HF], bf16, isOutput=False)
    woT = nc.declare_dram_parameter("woT", [HF, D], bf16, isOutput=False)
    bqd = nc.declare_dram_parameter("bq", [HF], f32, isOutput=False)
    bkd = nc.declare_dram_parameter("bk", [HF], f32, isOutput=False)
    bvd = nc.declare_dram_parameter("bv", [HF], bf16, isOutput=False)
    out = nc.declare_dram_parameter("out", [S, D], f32, isOutput=True)
    identd = nc.declare_dram_parameter("ident", [PT, PT], bf16, isOutput=False)

    DKT = D // PT       # 8 feature k-tiles for projections
    QC2 = 256           # q-chunk for attention
    NQC2 = S // QC2     # 8
    KG = 2              # k-tiles per score/exp group (1 PSUM bank per tile)
    NG = KT // KG       # 8 groups

    with tile.TileContext(nc) as tc, ExitStack() as ctx:
        persist = ctx.enter_context(tc.tile_pool(name="persist", bufs=1))
        xpool = ctx.enter_context(tc.tile_pool(name="p1x", bufs=2))
        atpool = ctx.enter_context(tc.tile_pool(name="at", bufs=2))
        nrmpool = ctx.enter_context(tc.tile_pool(name="nrm", bufs=2))
        cspool = ctx.enter_context(tc.tile_pool(name="cs", bufs=2))
        opool = ctx.enter_context(tc.tile_pool(name="p3o", bufs=2))
        qkpool = ctx.enter_context(tc.tile_pool(name="qk", bufs=2))
        pspool = ctx.enter_context(tc.tile_pool(name="ps", bufs=2, space="PSUM"))
        stpool = ctx.enter_context(tc.tile_pool(name="st", bufs=1, space="PSUM"))
        avpool = ctx.enter_context(tc.tile_pool(name="av", bufs=1, space="PSUM"))
        p3pool = ctx.enter_context(tc.tile_pool(name="p3", bufs=1, space="PSUM"))

        # ---- persistent state: weights, biases, constants --------------
        ctxT = [[persist.tile([PT, PT], bf16, name=f"ctxT{i}_{t}", tag=f"ctxT{i}_{t}")
                 for t in range(KT)] for i in range(4)]
        bvb = persist.tile([PT, HF], bf16, tag="bvb")
        ident = persist.tile([PT, PT], bf16, tag="ident")
        nc.sync.dma_start(ident[:], identd[:])

        bv_ap = bvd[:]
        bv_bc_src = bass.AP(
            tensor=bv_ap.tensor, offset=bv_ap.offset, ap=[[0, PT], [1, HF]]
        )
        nc.sync.dma_start(bvb[:], bv_bc_src)

        wts = {}
        for pname, wT_d in (("v", wvT), ("k", wkT), ("q", wqT)):
            wt = persist.tile([PT, DKT, HF], bf16, name=f"w{pname}", tag=f"w{pname}")
            nc.sync.dma_start(
                wt[:], wT_d.rearrange("(k p) f -> p k f", p=PT)
            )
            wts[pname] = [wt[:, k, :] for k in range(DKT)]
        bts = {}
        for pname, b_d in (("q", bqd), ("k", bkd)):
            bt = persist.tile([PT, 4], f32, name=f"b{pname}", tag=f"b{pname}")
            nc.sync.dma_start(bt[:], b_d.rearrange("(m p) -> p m", p=PT))
            bts[pname] = [bt[:, m : m + 1] for m in range(4)]
        woTa = persist.tile([PT, 4, D], bf16, name="woT", tag="woT")
        nc.sync.dma_start(woTa[:], woT.rearrange("(i p) d -> p i d", p=PT))
        woTt = [woTa[:, i, :] for i in range(4)]

        def alloc_rep():
            # fresh double-buffered q^T/k^T/v per rep so the next rep's
            # projections never WAR-block on this rep's score reads
            qT = [[qkpool.tile([PT, QC], bf16, name=f"qT{i}_{c}", tag=f"qT{i}_{c}")
                   for c in range(NQC)] for i in range(4)]
            kTt = [[qkpool.tile([PT, QC], bf16, name=f"kT{i}_{c}", tag=f"kT{i}_{c}")
                    for c in range(NQC)] for i in range(4)]
            vst = [qkpool.tile([PT, HPC * (HD + 1)], bf16, name=f"v{t}", tag=f"v{t}")
                   for t in range(KT)]
            for t in range(KT):
                v3 = vst[t].rearrange("p (h c) -> p h c", c=HD + 1)
                nc.gpsimd.memset(v3[:, :, HD : HD + 1], 1.0)
            return qT, kTt, vst

        def make_proj_tasks(qT, kTt, vst):
            """One rep's projections as closures: one DMA or one PSUM chain
            each, emitted one-per-beat inside the PREVIOUS rep's unit stream
            so the proj phase never runs serial."""
            tasks = []
            stage = {}
            nkp = 1 if "proj" in ablate else DKT

            def dma_task(xT_d, c, key):
                def f():
                    xta = xpool.tile([PT, DKT, QC], bf16, name="x", tag="x")
                    nc.sync.dma_start(
                        xta[:],
                        xT_d.rearrange("(k p) s -> p k s", p=PT)[:, :, c * QC : (c + 1) * QC],
                    )
                    stage[key] = xta
                return f

            def kq_chain(pname, c, m, key):
                def f():
                    xta = stage[key]
                    ps = pspool.tile([PT, QC], f32, tag="ps")
                    for k in range(nkp):
                        nc.tensor.matmul(
                            ps[:],
                            lhsT=wts[pname][k][:, m * PT : (m + 1) * PT],
                            rhs=xta[:, k, :],
                            start=(k == 0),
                            stop=(k == nkp - 1),
                        )
                    dstT = kTt if pname == "k" else qT
                    nc.vector.tensor_scalar_add(
                        dstT[m][c][:], ps[:], bts[pname][m][:]
                    )
                return f

            def v_chain(c, mt, key):
                def f():
                    xta = stage[key]
                    t = c * 4 + mt
                    ps = pspool.tile([PT, HF], f32, tag="ps")
                    for k in range(nkp):
                        nc.tensor.matmul(
                            ps[:],
                            lhsT=xta[:, k, mt * PT : (mt + 1) * PT],
                            rhs=wts["v"][k][:],
                            start=(k == 0),
                            stop=(k == nkp - 1),
                        )
                    v3 = vst[t].rearrange("p (h c) -> p h c", c=HD + 1)
                    nc.vector.tensor_add(
                        v3[:, :, 0:HD],
                        ps[:].rearrange("p (h c) -> p h c", c=HD),
                        bvb[:].rearrange("p (h c) -> p h c", c=HD),
                    )
                return f

            def add_kq(c):
                for pname, xT_d in (("k", xkT), ("q", xqT)):
                    key = (pname, c)
                    tasks.append(dma_task(xT_d, c, key))
                    for m in range(4):
                        tasks.append(kq_chain(pname, c, m, key))

            def add_v(c):
                key = ("v", c)
                tasks.append(dma_task(xvT, c, key))
                for mt in range(4):
                    tasks.append(v_chain(c, mt, key))

            add_kq(0)
            add_kq(1)
            for c in range(NQC):
                add_v(c)
            add_kq(2)
            add_kq(3)
            return tasks

        def emit_units(qT, kTt, vst, feed):
            # ---- phase 2+3: software-pipelined units ---------------------
            # unit u = (c, hp). Emission interleaves scores/exp of unit u with
            # the AV chains of unit u-1 in 4 "beats", so PE always has runnable
            # matmuls while Act/DVE drain the previous unit's exps. Score
            # tiles live in 4 single-bank parity slots st{e}p{g%2}; at tiles
            # have 16 tags x 2 bufs. The NEXT rep's projection tasks are fed
            # one per beat starting at unit FEED_START.
            at_store = {}
            cs_store = {}
            exp_ctr = [0]

            # One [128, 1024] score tile per group spanning TWO banks: e0 in
            # cols 0:512 (bank A), e1 in 512:1024 (bank B) - the alternating
            # e matmuls still hit different banks (keeps the PE row-tile
            # concurrency) while exp becomes ONE [128,1024] instruction per
            # group, halving the per-instruction fixed overhead on Act/DVE.
            def emit_scores(c, hp, g):
                stp = stpool.tile([PT, 2 * KG * QC2], f32, name=f"stp{g % 2}",
                                  tag=f"stp{g % 2}")
                scols = 16 if "scores" in ablate else QC2
                for j in range(KG):
                    kt = KG * g + j
                    for e in range(2):
                        nc.tensor.matmul(
                            stp[:, e * KG * QC2 + j * QC2 :
                                e * KG * QC2 + j * QC2 + scols],
                            lhsT=kTt[hp][kt // 4][e * HD : (e + 1) * HD,
                                                  (kt % 4) * PT : (kt % 4 + 1) * PT],
                            rhs=qT[hp][c // 2][e * HD : (e + 1) * HD,
                                               (c % 2) * QC2 : (c % 2) * QC2 + scols],
                            start=True,
                            stop=True,
                        )
                a = atpool.tile([PT, 2 * KG * QC2], bf16,
                                name=f"at{g}", tag=f"at{g}")
                i = exp_ctr[0]
                exp_ctr[0] += 1
                ecols = 64 if "exp" in ablate else 2 * KG * QC2
                if (i * ACT_N) % EXP_PERIOD < ACT_N:
                    nc.scalar.activation(
                        a[:, 0:ecols], stp[:, 0:ecols],
                        mybir.ActivationFunctionType.Exp,
                        scale=1.0 / np.sqrt(HD),
                    )
                else:
                    nc.vector.tensor_scalar(
                        a[:, 0:ecols].bitcast(i16), stp[:, 0:ecols],
                        C1, C2, ALU.mult, ALU.add,
                    )
                at_store[c, hp, g] = a

            def av_chain(c, hp, qt, e):
                # One [128, 260] PSUM tile per unit; the 4 chains use disjoint
                # 65-col regions, so subtile dep-tracking lets chain k+1
                # accumulate while DVE still normalizes chain k (no PE stall),
                # and PSUM accumulation state is per-address so interleaved
                # groups on disjoint regions are safe.
                if (c, hp, 0) not in cs_store:
                    for q2 in range(2):
                        cs_store[c, hp, q2] = cspool.tile(
                            [PT, PT], bf16, name=f"cs{hp}_{q2}", tag=f"cs{hp}_{q2}")
                    cs_store[c, hp, "ave"] = avpool.tile(
                        [PT, 4 * (HD + 1)], f32, name="av", tag="ave")
                ci = 2 * qt + e
                avp = cs_store[c, hp, "ave"][:, ci * (HD + 1) : (ci + 1) * (HD + 1)]
                h = 2 * hp + e
                nkt = 2 if "av" in ablate else KT
                for kt in range(nkt):
                    g, j = kt // KG, kt % KG
                    last = j == KG - 1 and qt == 1 and e == 1 and nkt == KT
                    at = at_store.pop((c, hp, g)) if last else at_store[c, hp, g]
                    off = e * KG * QC2 + j * QC2 + qt * PT
                    nc.tensor.matmul(
                        avp,
                        lhsT=at[:, off : off + PT],
                        rhs=vst[kt][:, h * (HD + 1) : (h + 1) * (HD + 1)],
                        start=(kt == 0),
                        stop=(kt == nkt - 1),
                    )
                linv = nrmpool.tile([PT, 1], f32, tag="linv")
                nc.vector.reciprocal(linv[:], avp[:, HD : HD + 1])
                nc.vector.tensor_scalar_mul(
                    cs_store[c, hp, qt][:, e * HD : (e + 1) * HD],
                    avp[:, 0:HD],
                    linv[:],
                )
                if ci == 3:
                    cs_store.pop((c, hp, "ave"))

            def emit_tps(c, hp):
                for qt in range(2):
                    tt = 2 * c + qt
                    tp = p3pool.tile([PT, PT], bf16, name="tp", tag="p3")
                    nc.tensor.transpose(tp[:], cs_store.pop((c, hp, qt))[:], ident[:])
                    nc.vector.tensor_copy(ctxT[hp][tt][:], tp[:])

            def emit_outproj(c):
                for qt in range(2):
                    tt = 2 * c + qt
                    for nch in range(2):
                        ps = p3pool.tile([PT, QC], f32, tag="p3")
                        nko = 1 if "outproj" in ablate else 4
                        for k in range(nko):
                            nc.tensor.matmul(
                                ps[:],
                                lhsT=ctxT[k][tt][:],
                                rhs=woTt[k][:, nch * QC : (nch + 1) * QC],
                                start=(k == 0),
                                stop=(k == nko - 1),
                            )
                        ot = opool.tile([PT, QC], f32, tag="ot")
                        nc.vector.tensor_copy(ot[:], ps[:])
                        nc.sync.dma_start(
                            out[tt * PT : (tt + 1) * PT, nch * QC : (nch + 1) * QC], ot[:]
                        )

            FEED_START = 2 if FEED == "b2" else 8
            pace = 2 if FEED == "b2" else 1
            bctr = [0]
            units = [(c, hp) for c in range(NQC2) for hp in range(4)]
            for ui in range(len(units) + 1):
                cur = units[ui] if ui < len(units) else None
                prev = units[ui - 1] if ui >= 1 else None
                for beat in range(4):
                    if cur is not None:
                        emit_scores(cur[0], cur[1], 2 * beat)
                        emit_scores(cur[0], cur[1], 2 * beat + 1)
                    if prev is not None:
                        av_chain(prev[0], prev[1], beat // 2, beat % 2)
                    if feed and ui >= FEED_START:
                        bctr[0] += 1
                        if bctr[0] % pace == 0:
                            feed.pop(0)()
                if prev is not None:
                    emit_tps(prev[0], prev[1])
                    if prev[1] == 3:
                        emit_outproj(prev[0])

        cur_tiles = alloc_rep()
        cur_tasks = make_proj_tasks(*cur_tiles)
        for t in cur_tasks:  # rep 0 prologue: projections inline
            t()
        for _rep in range(reps):
            if _rep + 1 < reps:
                nxt_tiles = alloc_rep()
                nxt_tasks = make_proj_tasks(*nxt_tiles)
            else:
                nxt_tiles, nxt_tasks = None, []
            emit_units(*cur_tiles, feed=(nxt_tasks if FEED != "off" else []))
            for t in nxt_tasks:  # leftovers / FEED=off: proj at rep start
                t()
            cur_tiles = nxt_tiles

    nc.compile()
    return nc


def make_in_maps(inputs):
    import ml_dtypes

    q = np.ascontiguousarray(inputs["query"], dtype=np.float32)
    k = np.ascontiguousarray(inputs["key"], dtype=np.float32)
    v = np.ascontiguousarray(inputs["value"], dtype=np.float32)
    Wq, Wk, Wv, Wo = (np.asarray(inputs[n], np.float32) for n in ("Wq", "Wk", "Wv", "Wo"))
    bq, bk, bv, bo = (np.asarray(inputs[n], np.float32) for n in ("bq", "bk", "bv", "bo"))

    in_maps = []
    for c in range(NCORES):
        b, half = c // 2, c % 2
        fs = slice(half * HF, (half + 1) * HF)
        in_maps.append({
            "xqT": np.ascontiguousarray(q[b].T).astype(ml_dtypes.bfloat16),
            "xkT": np.ascontiguousarray(k[b].T).astype(ml_dtypes.bfloat16),
            "xvT": np.ascontiguousarray(v[b].T).astype(ml_dtypes.bfloat16),
            "wqT": np.ascontiguousarray(Wq[fs, :].T).astype(ml_dtypes.bfloat16),
            "wkT": np.ascontiguousarray(Wk[fs, :].T).astype(ml_dtypes.bfloat16),
            "wvT": np.ascontiguousarray(Wv[fs, :].T).astype(ml_dtypes.bfloat16),
            "woT": np.ascontiguousarray(Wo[:, fs].T).astype(ml_dtypes.bfloat16),
            "bq": np.ascontiguousarray(bq[fs]),
            "bk": np.ascontiguousarray(bk[fs]),
            "bv": np.ascontiguousarray(bv[fs]).astype(ml_dtypes.bfloat16),
            "ident": np.eye(PT, dtype=ml_dtypes.bfloat16),
        })
    return in_maps


def kernel(**inputs):
    from concourse.bass_utils import run_bass_kernel_spmd

    if "nc" not in _cache:
        _cache["nc"] = _build_nc()
    nc = _cache["nc"]

    in_maps = make_in_maps(inputs)
    res = run_bass_kernel_spmd(nc, in_maps, list(range(NCORES)))
    _cache["last_result"] = res

    bo = np.asarray(inputs["bo"], np.float32)
    out = np.empty((B, S, D), np.float32)
    for b in range(B):
        out[b] = res.results[2 * b]["out"] + res.results[2 * b + 1]["out"] + bo
    return out



# revision 34
# speedup vs baseline: 1.1576x; 1.0804x over previous
"""Multihead attention (B=4, S=2048, D=1024, H=16, Hd=64) on 8 trn2 cores.

Sharding: core c owns batch b = c//2 and heads [(c%2)*8, (c%2)*8+8).
Each core computes q/k/v projections for its 8 heads, attention, and the
partial output projection restricted to its heads' context features.
Host adds the two partials per batch element (+ bo).

Dtype strategy (the error gate is 2e-2; measured rel err ~6e-3):
  - x and all weights are converted to bf16 on the host; every matmul is
    bf16 x bf16 -> f32 PSUM, which runs at 1 PE-cycle per output column
    (fp32 runs at 4) at ANY moving width - that matters for the narrow
    (N=65) AV matmuls. bf16 also halves input DMA and SBUF footprint.
  - exp runs on the Act engine reading f32 PSUM scores, writing bf16 A^T
    in [128, 1024] instructions (KG=4 k-tiles per group) to amortize the
    ~185ns per-instruction access latency.

Pipeline (the graded metric is the marginal per-rep time, i.e. the
steady-state pipeline period):
  - all tile pools and weight/bias/const DMAs sit OUTSIDE the rep loop;
    consecutive reps overlap through WAR dependencies on rotating tiles.
  - q^T/k^T/vst are double-buffered (qk pool, bufs=2) so rep r+1's
    projections run inside rep r's attention window instead of blocking
    on its score/AV reads. Projections run k/q interleaved per chunk
    (per-chunk q^T/k^T tiles keep the dependencies fine-grained), then v.
  - PSUM (8 banks): proj "ps" x2 = 2, score tiles st0/st1 (2 banks each,
    KG*QC2 f32 cols) = 4, AV "ave" = 1, out-proj "p3" = 1. AV chains run
    sequentially on one bank so phase 3 gets a dedicated bank and never
    blocks the next rep's attention; the ctx transposes rotate on the p3
    bank (end-of-section work, no cross-rep hazard, and off the AV
    rotation). Four interleaved accumulation chains must NOT share one
    bank (corrupts accumulation), hence one chain at a time.
  - steady state is PE-bound: per-rep PE ~281us busy (proj 109 + scores
    109 (K=64: half-array, intrinsic to Hd=64) + AV 55 + out-proj 27);
    sim marginal ~300us/rep vs ~1130us for the fp32 baseline.

Layout:
  - inputs are fed pre-transposed (xT: [D, S]) so projection matmuls need
    no on-device transposes.
  - q, k are produced transposed ([hd, tok]); scores are computed as
    S^T = K @ Q^T with k-tokens on partitions so the exp output A^T is
    already in the layout the AV matmul needs as its stationary operand.
    Head pairs share the PE array rows (even head rows 0-63, odd 64-127).
  - AV runs with the narrow [V | 1] operand moving (N=65): out[q, 0:64] is
    the context, out[q, 64] the softmax denominator, so normalization is a
    per-partition reciprocal+scale. ctx tiles are PE-transposed into ctx^T
    for the output projection.
  - softmax skips max-subtraction: scores are ~N(0,1) here, exp is safe
    and matches the max-subtracted reference to rounding error.
"""

import numpy as np

B, S, D = 4, 2048, 1024
H, HD = 16, 64
HPC = 8              # heads per core
HF = HPC * HD        # 512 head-features per core
NCORES = 8
QC = 512             # query-chunk (matmul moving free dim)
NQC = S // QC        # 4
KT = S // 128        # 16 k-token tiles
PT = 128

_cache = {}

# exp engine split: the 256 per-rep exp instructions are distributed between
# the Act engine (native Exp, ~893ns/[128,1024]) and DVE (Schraudolph bf16-bits
# exp via tensor_scalar -> int16, ~1095ns, rms rel err ~1.8% which softmax
# normalization damps by ~sqrt(N_eff)~27x - negligible vs the 2e-2 gate).
# GPSIMD cannot read PSUM, so Pool only takes SBUF-side copies/memsets.
# Ratio ACT_N/EXP_PERIOD balances Act vs DVE (DVE also carries ~70us of
# normalization/bias work per rep).
EXP_PERIOD = 16
ACT_N = 11
SIGMA = 0.0580
FEED = "b1"   # "off" | "b1" (task/beat) | "b2" (task per 2 beats)


def _build_nc(reps=1, ablate=()):
    """ablate: timing-only stage shrinks ("exp","scores","av","proj",
    "outproj") that keep the dependency structure but cut ~90% of one
    stage's engine work. Never used by kernel() - localizes bottlenecks."""
    from contextlib import ExitStack

    import concourse.mybir as mybir
    import concourse.tile as tile
    from concourse import bacc
    import concourse.bass as bass

    f32 = mybir.dt.float32
    bf16 = mybir.dt.bfloat16
    i16 = mybir.dt.int16
    ALU = mybir.AluOpType
    C1 = float(np.log2(np.e) / np.sqrt(HD) * 128.0)
    C2 = float((127.0 - SIGMA) * 128.0)
    nc = bacc.Bacc()

    xqT = nc.declare_dram_parameter("xqT", [D, S], bf16, isOutput=False)
    xkT = nc.declare_dram_parameter("xkT", [D, S], bf16, isOutput=False)
    xvT = nc.declare_dram_parameter("xvT", [D, S], bf16, isOutput=False)
    wqT = nc.declare_dram_parameter("wqT", [D, HF], bf16, isOutput=False)
    wkT = nc.declare_dram_parameter("wkT", [D, HF], bf16, isOutput=False)
    wvT = nc.declare_dram_parameter("wvT", [D, HF], bf16, isOutput=False)
    woT = nc.declare_dram_parameter("woT", [HF, D], bf16, isOutput=False)
    bqd = nc.declare_dram_parameter("bq", [HF], f32, isOutput=False)
    bkd = nc.declare_dram_parameter("bk", [HF], f32, isOutput=False)
    bvd = nc.declare_dram_parameter("bv", [HF], bf16, isOutput=False)
    out = nc.declare_dram_parameter("out", [S, D], bf16, isOutput=True)
    identd = nc.declare_dram_parameter("ident", [PT, PT], bf16, isOutput=False)

    DKT = D // PT       # 8 feature k-tiles for projections
    QC2 = 256           # q-chunk for attention
    NQC2 = S // QC2     # 8
    KG = 2              # k-tiles per score/exp group (1 PSUM bank per tile)
    NG = KT // KG       # 8 groups

    with tile.TileContext(nc) as tc, ExitStack() as ctx:
        persist = ctx.enter_context(tc.tile_pool(name="persist", bufs=1))
        xpool = ctx.enter_context(tc.tile_pool(name="p1x", bufs=2))
        atpool = ctx.enter_context(tc.tile_pool(name="at", bufs=2))
        nrmpool = ctx.enter_context(tc.tile_pool(name="nrm", bufs=2))
        cspool = ctx.enter_context(tc.tile_pool(name="cs", bufs=2))
        opool = ctx.enter_context(tc.tile_pool(name="p3o", bufs=2))
        qkpool = ctx.enter_context(tc.tile_pool(name="qk", bufs=2))
        pspool = ctx.enter_context(tc.tile_pool(name="ps", bufs=2, space="PSUM"))
        stpool = ctx.enter_context(tc.tile_pool(name="st", bufs=1, space="PSUM"))
        avpool = ctx.enter_context(tc.tile_pool(name="av", bufs=1, space="PSUM"))
        p3pool = ctx.enter_context(tc.tile_pool(name="p3", bufs=1, space="PSUM"))

        # ---- persistent state: weights, biases, constants --------------
        ctxT = [[persist.tile([PT, PT], bf16, name=f"ctxT{i}_{t}", tag=f"ctxT{i}_{t}")
                 for t in range(KT)] for i in range(4)]
        bvb = persist.tile([PT, HF], bf16, tag="bvb")
        ident = persist.tile([PT, PT], bf16, tag="ident")
        nc.sync.dma_start(ident[:], identd[:])

        bv_ap = bvd[:]
        bv_bc_src = bass.AP(
            tensor=bv_ap.tensor, offset=bv_ap.offset, ap=[[0, PT], [1, HF]]
        )
        nc.sync.dma_start(bvb[:], bv_bc_src)

        wts = {}
        for pname, wT_d in (("v", wvT), ("k", wkT), ("q", wqT)):
            wt = persist.tile([PT, DKT, HF], bf16, name=f"w{pname}", tag=f"w{pname}")
            nc.sync.dma_start(
                wt[:], wT_d.rearrange("(k p) f -> p k f", p=PT)
            )
            wts[pname] = [wt[:, k, :] for k in range(DKT)]
        bts = {}
        for pname, b_d in (("q", bqd), ("k", bkd)):
            bt = persist.tile([PT, 4], f32, name=f"b{pname}", tag=f"b{pname}")
            nc.sync.dma_start(bt[:], b_d.rearrange("(m p) -> p m", p=PT))
            bts[pname] = [bt[:, m : m + 1] for m in range(4)]
        woTa = persist.tile([PT, 4, D], bf16, name="woT", tag="woT")
        nc.sync.dma_start(woTa[:], woT.rearrange("(i p) d -> p i d", p=PT))
        woTt = [woTa[:, i, :] for i in range(4)]

        def alloc_rep():
            # fresh double-buffered q^T/k^T/v per rep so the next rep's
            # projections never WAR-block on this rep's score reads
            qT = [[qkpool.tile([PT, QC], bf16, name=f"qT{i}_{c}", tag=f"qT{i}_{c}")
                   for c in range(NQC)] for i in range(4)]
            kTt = [[qkpool.tile([PT, QC], bf16, name=f"kT{i}_{c}", tag=f"kT{i}_{c}")
                    for c in range(NQC)] for i in range(4)]
            vst = [qkpool.tile([PT, HPC * (HD + 1)], bf16, name=f"v{t}", tag=f"v{t}")
                   for t in range(KT)]
            for t in range(KT):
                v3 = vst[t].rearrange("p (h c) -> p h c", c=HD + 1)
                nc.gpsimd.memset(v3[:, :, HD : HD + 1], 1.0)
            return qT, kTt, vst

        def make_proj_tasks(qT, kTt, vst):
            """One rep's projections as closures: one DMA or one PSUM chain
            each, emitted one-per-beat inside the PREVIOUS rep's unit stream
            so the proj phase never runs serial."""
            tasks = []
            stage = {}
            nkp = 1 if "proj" in ablate else DKT

            def dma_task(xT_d, c, key):
                def f():
                    xta = xpool.tile([PT, DKT, QC], bf16, name="x", tag="x")
                    nc.sync.dma_start(
                        xta[:],
                        xT_d.rearrange("(k p) s -> p k s", p=PT)[:, :, c * QC : (c + 1) * QC],
                    )
                    stage[key] = xta
                return f

            def kq_chain(pname, c, m, key):
                def f():
                    xta = stage[key]
                    ps = pspool.tile([PT, QC], f32, tag="ps")
                    for k in range(nkp):
                        nc.tensor.matmul(
                            ps[:],
                            lhsT=wts[pname][k][:, m * PT : (m + 1) * PT],
                            rhs=xta[:, k, :],
                            start=(k == 0),
                            stop=(k == nkp - 1),
                        )
                    dstT = kTt if pname == "k" else qT
                    nc.vector.tensor_scalar_add(
                        dstT[m][c][:], ps[:], bts[pname][m][:]
                    )
                return f

            def v_chain(c, mt, key):
                def f():
                    xta = stage[key]
                    t = c * 4 + mt
                    ps = pspool.tile([PT, HF], f32, tag="ps")
                    for k in range(nkp):
                        nc.tensor.matmul(
                            ps[:],
                            lhsT=xta[:, k, mt * PT : (mt + 1) * PT],
                            rhs=wts["v"][k][:],
                            start=(k == 0),
                            stop=(k == nkp - 1),
                        )
                    v3 = vst[t].rearrange("p (h c) -> p h c", c=HD + 1)
                    nc.vector.tensor_add(
                        v3[:, :, 0:HD],
                        ps[:].rearrange("p (h c) -> p h c", c=HD),
                        bvb[:].rearrange("p (h c) -> p h c", c=HD),
                    )
                return f

            def add_kq(c):
                for pname, xT_d in (("k", xkT), ("q", xqT)):
                    key = (pname, c)
                    tasks.append(dma_task(xT_d, c, key))
                    for m in range(4):
                        tasks.append(kq_chain(pname, c, m, key))

            def add_v(c):
                key = ("v", c)
                tasks.append(dma_task(xvT, c, key))
                for mt in range(4):
                    tasks.append(v_chain(c, mt, key))

            add_kq(0)
            add_kq(1)
            for c in range(NQC):
                add_v(c)
            add_kq(2)
            add_kq(3)
            return tasks

        def emit_units(qT, kTt, vst, feed):
            # ---- phase 2+3: software-pipelined units ---------------------
            # unit u = (c, hp). Emission interleaves scores/exp of unit u with
            # the AV chains of unit u-1 in 4 "beats", so PE always has runnable
            # matmuls while Act/DVE drain the previous unit's exps. Score
            # tiles live in 4 single-bank parity slots st{e}p{g%2}; at tiles
            # have 16 tags x 2 bufs. The NEXT rep's projection tasks are fed
            # one per beat starting at unit FEED_START.
            at_store = {}
            cs_store = {}
            exp_ctr = [0]

            # One [128, 1024] score tile per group spanning TWO banks: e0 in
            # cols 0:512 (bank A), e1 in 512:1024 (bank B) - the alternating
            # e matmuls still hit different banks (keeps the PE row-tile
            # concurrency) while exp becomes ONE [128,1024] instruction per
            # group, halving the per-instruction fixed overhead on Act/DVE.
            def emit_scores(c, hp, g):
                stp = stpool.tile([PT, 2 * KG * QC2], f32, name=f"stp{g % 2}",
                                  tag=f"stp{g % 2}")
                scols = 16 if "scores" in ablate else QC2
                for j in range(KG):
                    kt = KG * g + j
                    for e in range(2):
                        nc.tensor.matmul(
                            stp[:, e * KG * QC2 + j * QC2 :
                                e * KG * QC2 + j * QC2 + scols],
                            lhsT=kTt[hp][kt // 4][e * HD : (e + 1) * HD,
                                                  (kt % 4) * PT : (kt % 4 + 1) * PT],
                            rhs=qT[hp][c // 2][e * HD : (e + 1) * HD,
                                               (c % 2) * QC2 : (c % 2) * QC2 + scols],
                            start=True,
                            stop=True,
                        )
                a = atpool.tile([PT, 2 * KG * QC2], bf16,
                                name=f"at{g}", tag=f"at{g}")
                i = exp_ctr[0]
                exp_ctr[0] += 1
                ecols = 64 if "exp" in ablate else 2 * KG * QC2
                if (i * ACT_N) % EXP_PERIOD < ACT_N:
                    nc.scalar.activation(
                        a[:, 0:ecols], stp[:, 0:ecols],
                        mybir.ActivationFunctionType.Exp,
                        scale=1.0 / np.sqrt(HD),
                    )
                else:
                    nc.vector.tensor_scalar(
                        a[:, 0:ecols].bitcast(i16), stp[:, 0:ecols],
                        C1, C2, ALU.mult, ALU.add,
                    )
                at_store[c, hp, g] = a

            def av_chain(c, hp, qt, e):
                # One [128, 260] PSUM tile per unit; the 4 chains use disjoint
                # 65-col regions, so subtile dep-tracking lets chain k+1
                # accumulate while DVE still normalizes chain k (no PE stall),
                # and PSUM accumulation state is per-address so interleaved
                # groups on disjoint regions are safe.
                if (c, hp, 0) not in cs_store:
                    for q2 in range(2):
                        cs_store[c, hp, q2] = cspool.tile(
                            [PT, PT], bf16, name=f"cs{hp}_{q2}", tag=f"cs{hp}_{q2}")
                    cs_store[c, hp, "ave"] = avpool.tile(
                        [PT, 4 * (HD + 1)], f32, name="av", tag="ave")
                ci = 2 * qt + e
                avp = cs_store[c, hp, "ave"][:, ci * (HD + 1) : (ci + 1) * (HD + 1)]
                h = 2 * hp + e
                nkt = 2 if "av" in ablate else KT
                for kt in range(nkt):
                    g, j = kt // KG, kt % KG
                    last = j == KG - 1 and qt == 1 and e == 1 and nkt == KT
                    at = at_store.pop((c, hp, g)) if last else at_store[c, hp, g]
                    off = e * KG * QC2 + j * QC2 + qt * PT
                    nc.tensor.matmul(
                        avp,
                        lhsT=at[:, off : off + PT],
                        rhs=vst[kt][:, h * (HD + 1) : (h + 1) * (HD + 1)],
                        start=(kt == 0),
                        stop=(kt == nkt - 1),
                    )
                linv = nrmpool.tile([PT, 1], f32, tag="linv")
                nc.vector.reciprocal(linv[:], avp[:, HD : HD + 1])
                nc.vector.tensor_scalar_mul(
                    cs_store[c, hp, qt][:, e * HD : (e + 1) * HD],
                    avp[:, 0:HD],
                    linv[:],
                )
                if ci == 3:
                    cs_store.pop((c, hp, "ave"))

            def emit_tps(c, hp):
                for qt in range(2):
                    tt = 2 * c + qt
                    tp = p3pool.tile([PT, PT], bf16, name="tp", tag="p3")
                    nc.tensor.transpose(tp[:], cs_store.pop((c, hp, qt))[:], ident[:])
                    nc.vector.tensor_copy(ctxT[hp][tt][:], tp[:])

            def emit_outproj(c):
                for qt in range(2):
                    tt = 2 * c + qt
                    for nch in range(2):
                        ps = p3pool.tile([PT, QC], f32, tag="p3")
                        nko = 1 if "outproj" in ablate else 4
                        for k in range(nko):
                            nc.tensor.matmul(
                                ps[:],
                                lhsT=ctxT[k][tt][:],
                                rhs=woTt[k][:, nch * QC : (nch + 1) * QC],
                                start=(k == 0),
                                stop=(k == nko - 1),
                            )
                        ot = opool.tile([PT, QC], bf16, tag="ot")
                        nc.vector.tensor_copy(ot[:], ps[:])
                        nc.sync.dma_start(
                            out[tt * PT : (tt + 1) * PT, nch * QC : (nch + 1) * QC], ot[:]
                        )

            FEED_START = 2 if FEED == "b2" else 8
            pace = 2 if FEED == "b2" else 1
            bctr = [0]
            units = [(c, hp) for c in range(NQC2) for hp in range(4)]
            for ui in range(len(units) + 1):
                cur = units[ui] if ui < len(units) else None
                prev = units[ui - 1] if ui >= 1 else None
                for beat in range(4):
                    if cur is not None:
                        emit_scores(cur[0], cur[1], 2 * beat)
                        emit_scores(cur[0], cur[1], 2 * beat + 1)
                    if prev is not None:
                        av_chain(prev[0], prev[1], beat // 2, beat % 2)
                    if feed and ui >= FEED_START:
                        bctr[0] += 1
                        if bctr[0] % pace == 0:
                            feed.pop(0)()
                if prev is not None:
                    emit_tps(prev[0], prev[1])
                    if prev[1] == 3:
                        emit_outproj(prev[0])

        cur_tiles = alloc_rep()
        cur_tasks = make_proj_tasks(*cur_tiles)
        for t in cur_tasks:  # rep 0 prologue: projections inline
            t()
        for _rep in range(reps):
            if _rep + 1 < reps:
                nxt_tiles = alloc_rep()
                nxt_tasks = make_proj_tasks(*nxt_tiles)
            else:
                nxt_tiles, nxt_tasks = None, []
            emit_units(*cur_tiles, feed=(nxt_tasks if FEED != "off" else []))
            for t in nxt_tasks:  # leftovers / FEED=off: proj at rep start
                t()
            cur_tiles = nxt_tiles

    nc.compile()
    return nc


def make_in_maps(inputs):
    import ml_dtypes

    q = np.ascontiguousarray(inputs["query"], dtype=np.float32)
    k = np.ascontiguousarray(inputs["key"], dtype=np.float32)
    v = np.ascontiguousarray(inputs["value"], dtype=np.float32)
    Wq, Wk, Wv, Wo = (np.asarray(inputs[n], np.float32) for n in ("Wq", "Wk", "Wv", "Wo"))
    bq, bk, bv, bo = (np.asarray(inputs[n], np.float32) for n in ("bq", "bk", "bv", "bo"))

    in_maps = []
    for c in range(NCORES):
        b, half = c // 2, c % 2
        fs = slice(half * HF, (half + 1) * HF)
        in_maps.append({
            "xqT": np.ascontiguousarray(q[b].T).astype(ml_dtypes.bfloat16),
            "xkT": np.ascontiguousarray(k[b].T).astype(ml_dtypes.bfloat16),
            "xvT": np.ascontiguousarray(v[b].T).astype(ml_dtypes.bfloat16),
            "wqT": np.ascontiguousarray(Wq[fs, :].T).astype(ml_dtypes.bfloat16),
            "wkT": np.ascontiguousarray(Wk[fs, :].T).astype(ml_dtypes.bfloat16),
            "wvT": np.ascontiguousarray(Wv[fs, :].T).astype(ml_dtypes.bfloat16),
            "woT": np.ascontiguousarray(Wo[:, fs].T).astype(ml_dtypes.bfloat16),
            "bq": np.ascontiguousarray(bq[fs]),
            "bk": np.ascontiguousarray(bk[fs]),
            "bv": np.ascontiguousarray(bv[fs]).astype(ml_dtypes.bfloat16),
            "ident": np.eye(PT, dtype=ml_dtypes.bfloat16),
        })
    return in_maps


def kernel(**inputs):
    from concourse.bass_utils import run_bass_kernel_spmd

    if "nc" not in _cache:
        _cache["nc"] = _build_nc()
    nc = _cache["nc"]

    in_maps = make_in_maps(inputs)
    res = run_bass_kernel_spmd(nc, in_maps, list(range(NCORES)))
    _cache["last_result"] = res

    bo = np.asarray(inputs["bo"], np.float32)
    out = np.empty((B, S, D), np.float32)
    for b in range(B):
        out[b] = (res.results[2 * b]["out"].astype(np.float32)
                  + res.results[2 * b + 1]["out"].astype(np.float32) + bo)
    return out

